# revision 15
# baseline (speedup 1.0000x reference)
"""Trainium2 Bass kernel for nn_BiaffineSpan2WordLabeler.

Reference computation (B=4, L=128, IN=1024, H=512, NOUT=4):
    diff[b,i,j]  = x_const[b,j] - x_const[b,i]              # [B, L, L, IN]
    h1 = leaky(diff @ W1 + b1) * SCALE                      # [B, L*L, H]
    h2 = leaky(x_dep @ W2 + b2) * SCALE                     # [B, L, H]
    out[b,o,x,y] = sum_i h1b[b,x,i] Wa[o,i,j] h2[b,y,j]     # h1b = [h1, 1]

Algebraic restructurings (exact up to fp rounding):
  1. diff @ W1 = P[j] - P[i] where P = x_const @ W1; leaky applied after
     the elementwise assembly z[i,j] = P[j] - P[i] + b1.
  2. SCALE folded into W1,b1,W2,b2 host-side.
  3. Biaffine contracted as u[o,y,:] = Wa[o]·h2[y] first, then out = h1·u.

Sharding: 8 cores = (batch b = core//2) x (half of the i axis); SPMD via
host-side column permutation of x_const.

v6 (v4 structure + DMA-priority fixes; all costs measured from traces):
  * The prologue is INPUT-DMA-bound, not PE-bound (v5 lesson): all input
    DMAs ride the sync hw queue in strict priority order — tiny consts,
    then w1x (gates PT -> zg -> h1), then w2x, then wat — so nothing
    dilutes the critical transfer. The scalar queue carries no inputs.
  * PT/h2 as v1-style 128-row matmul chains (bf16 LDWEIGHTS ~107ns holds
    pacing near the DMA arrival rate; produces pts/h2t slabs directly,
    no transpose latency chain — v5's PTT->transpose->copy serial path
    delayed zg by 8us).
  * u-stage as 4x 512-row matmuls per o + PE transposes into ucat (53ns
    each; XBAR DMA transpose costs 1.2us queue time — never use).
  * zg/pts/nsneg FP32 (bf16 broadcast TT is 3x slower on DVE); zg
    tensor_tensors emitted separately from prelus (in-order queues);
    vector gets {0,4,8,11,14}, gpsimd (slow but idle) the rest.
  * Steady: [128,1024] PSUM pairs, 8 matmuls each, vector TT drain with
    fused +ubias, bf16 pair-DMA out on sync; final pair drained by
    scalar from ubias-seeded PSUM so the tail doesn't wait on vector.
"""

import sys

_REPO = "/opt/trn_rl_repo"
if _REPO not in sys.path:
    sys.path.insert(0, _REPO)

import os as _os

import numpy as np

B, L, IND, HID, NOUT = 4, 128, 1024, 512, 4
SCALE = 1.0 / (HID**0.25)
NCORES = 8
ILOC = 64  # i-values per core
KH = 4  # HID / 128
CIN = 8  # IND / 128
G = 4  # i-values per leaky group
NG = ILOC // G
VEC_ZG = {0, 3, 4, 8, 11, 14}  # zg groups on vector; rest on gpsimd
SC_PAIRS = {31}  # pairs drained by scalar (PSUM seeded with ubias)

_CACHED = {}


def _build_nc():
    import concourse.bass as bass
    import concourse.mybir as mybir
    from concourse.tile import TileContext
    import bass_rust

    F32 = mybir.dt.float32
    BF16 = mybir.dt.bfloat16
    AF = mybir.ActivationFunctionType
    ALU = mybir.AluOpType

    nc = bass.Bass()

    # [c*128+p, 0:512] = W1'[c*128+p, :], [c*128+p, 512:640] = xcT_perm[c*128+p, :]
    w1x = nc.dram_tensor("w1x", [IND, HID + L], BF16, kind="ExternalInput")
    w2x = nc.dram_tensor("w2x", [IND, HID + L], BF16, kind="ExternalInput")
    b1t = nc.dram_tensor("b1t", [128, KH], F32, kind="ExternalInput")
    b2t = nc.dram_tensor("b2t", [128, KH], F32, kind="ExternalInput")
    # wat[o][p, c*512 + i'] = Wa[o, i', c*128+p]   (i' < 512)
    wat = nc.dram_tensor("wat", [NOUT, 128, KH * HID], BF16, kind="ExternalInput")
    # wab[p, c*4+o] = Wa[o, 512, c*128+p]
    wab = nc.dram_tensor("wab", [128, KH * NOUT], BF16, kind="ExternalInput")
    idm = nc.dram_tensor("idm", [128, 128], BF16, kind="ExternalInput")
    out = nc.dram_tensor("out", [ILOC, L, NOUT * L], BF16, kind="ExternalOutput")

    with TileContext(nc) as tc:
        with (
            tc.tile_pool(name="constp", bufs=1) as constp,
            tc.tile_pool(name="wpool", bufs=3) as wpool,
            tc.tile_pool(name="watp", bufs=2) as watp,
            tc.tile_pool(name="pers", bufs=1) as pers,
            tc.tile_pool(name="work", bufs=6) as work,
            tc.tile_pool(name="h1pool", bufs=1) as h1pool,
            tc.tile_pool(name="outp", bufs=4) as outp,
            tc.tile_pool(name="ps1", bufs=2, space="PSUM") as ps1,
            tc.tile_pool(name="ps2", bufs=3, space="PSUM") as ps2,
        ):
            # ---- input DMAs: each queue's triggers serialize at ~680ns
            # (per-queue ~250GB/s), so spread the three big inputs across
            # the three DMA-capable queues: w1x/sync, w2x/scalar,
            # consts+wato/gpsimd ----
            wx_all = [wpool.tile([128, HID + L], BF16, name="wx", tag="wx", bufs=CIN) for _ in range(CIN)]
            for c in range(CIN):
                nc.sync.dma_start(wx_all[c], w1x[c * 128 : (c + 1) * 128, :])
            # separate tag: sharing w1x's tile rotation would make each w2x
            # DMA wait for PT to release the matching w1x chunk (v6.1 bug)
            w2_all = [wpool.tile([128, HID + L], BF16, name="w2", tag="w2", bufs=CIN) for _ in range(CIN)]
            for c in range(CIN):
                nc.scalar.dma_start(w2_all[c], w2x[c * 128 : (c + 1) * 128, :])
            b1t_sb = constp.tile([128, KH], F32)
            nc.gpsimd.dma_start(b1t_sb, b1t[:, :])
            b2t_sb = constp.tile([128, KH], F32)
            nc.gpsimd.dma_start(b2t_sb, b2t[:, :])
            wab_sb = constp.tile([128, KH * NOUT], BF16)
            nc.gpsimd.dma_start(wab_sb, wab[:, :])
            idm_sb = constp.tile([128, 128], BF16)
            nc.gpsimd.dma_start(idm_sb, idm[:, :])
            wato_all = [
                watp.tile([128, KH * HID], BF16, name="wato", bufs=NOUT)
                for _ in range(NOUT)
            ]
            for o in range(NOUT):
                nc.gpsimd.dma_start(wato_all[o], wat[o, :, :])

            ones_f = constp.tile([1, 128], F32)
            nc.vector.memset(ones_f, 1.0)
            ones_r = constp.tile([1, 128], BF16)
            nc.vector.tensor_copy(ones_r, ones_f)

            # ---- persistent intermediates ----
            pts = pers.tile([128, KH * L], F32)  # P: [h', (k, j)]
            nsneg = pers.tile([128, KH * ILOC], F32)  # P[:,i] - b1: [h', (k,i)]
            h2t = pers.tile([128, KH * L], BF16)  # h2^T: [j', (c, y)]
            u2 = [pers.tile([128, HID], BF16, name=f"u2_{o}") for o in range(NOUT)]
            ucat = [
                pers.tile([128, NOUT * L], BF16, name=f"ucat{k}") for k in range(KH)
            ]
            ubias_r = pers.tile([1, NOUT * L], BF16)
            ubias_bc = pers.tile([128, NOUT * L], F32)

            # ---- stage P: pts[h'_k, j] = sum_in W1'[in, h'_k] xcT[in, j] ----
            for k in range(KH):
                pspt = ps1.tile([128, NOUT * L], F32, name="ps", tag="ps")
                for c in range(CIN):
                    nc.tensor.matmul(
                        pspt[:, 0:L],
                        wx_all[c][:, k * 128 : (k + 1) * 128],
                        wx_all[c][:, HID : HID + L],
                        start=(c == 0),
                        stop=(c == CIN - 1),
                    )
                nc.vector.tensor_copy(pts[:, k * L : (k + 1) * L], pspt[:, 0:L])

            pts_kj = pts.rearrange("p (k j) -> p k j", k=KH)
            # per-k tensor_scalar (per-partition scalar operand) — the 3D
            # broadcast tensor_tensor form costs 2.3us on this chain
            for k in range(KH):
                nc.vector.tensor_scalar_sub(
                    nsneg[:, k * ILOC : (k + 1) * ILOC],
                    pts[:, k * L : k * L + ILOC],
                    b1t_sb[:, k : k + 1],
                )
            nsneg_ki = nsneg.rearrange("p (k i) -> p k i", k=KH)

            # ---- zg TTs (separate from prelus: in-order queues) ----
            h1_all = [
                h1pool.tile([128, G * KH * L], BF16, name=f"h1g{g}") for g in range(NG)
            ]
            zg_all = {}

            def emit_zg_tt(g):
                zg = work.tile([128, G * KH * L], F32, name="zg")
                zg_all[g] = zg
                z_eng = nc.vector if g in VEC_ZG else nc.gpsimd
                z_eng.tensor_tensor(
                    zg.rearrange("p (il k j) -> p il k j", il=G, k=KH),
                    pts_kj[:, None, :, :].to_broadcast((128, G, KH, L)),
                    nsneg_ki[:, :, g * G : (g + 1) * G]
                    .rearrange("p k i -> p i k")[:, :, :, None]
                    .to_broadcast((128, G, KH, L)),
                    ALU.subtract,
                )

            def emit_prelu(g):
                nc.scalar.activation(
                    h1_all[g], zg_all.pop(g), AF.Prelu, bias=0.0, scale=1.0, alpha=0.1
                )

            for g in range(4):
                emit_zg_tt(g)  # vector: 0; gpsimd: 1,2,3

            # ---- stage h2: h2t[j'_k, y] = leaky(sum_in W2'[in,j'_k] xdT[in,y] + b2) ----
            for k in range(KH):
                psh2 = ps1.tile([128, NOUT * L], F32, name="ps", tag="ps")
                for c in range(CIN):
                    nc.tensor.matmul(
                        psh2[:, 0:L],
                        w2_all[c][:, k * 128 : (k + 1) * 128],
                        w2_all[c][:, HID : HID + L],
                        start=(c == 0),
                        stop=(c == CIN - 1),
                    )
                nc.scalar.activation(
                    h2t[:, k * L : (k + 1) * L],
                    psh2[:, 0:L],
                    AF.Prelu,
                    bias=b2t_sb[:, k : k + 1],
                    scale=1.0,
                    alpha=0.1,
                )

            # ---- stage u (transposed): u2[o][y,h'] = sum_j' h2[y,j'] Wa[o,h',j']
            # then PE-transpose into ucat[k][h'_k, (o,y)] ----
            def emit_psu(o):
                psu = ps1.tile([128, HID], F32, name="ps", tag="ps")
                for c in range(KH):
                    nc.tensor.matmul(
                        psu,
                        h2t[:, c * L : (c + 1) * L],
                        wato_all[o][:, c * HID : (c + 1) * HID],
                        start=(c == 0),
                        stop=(c == KH - 1),
                    )
                nc.scalar.activation(u2[o], psu, AF.Copy, bias=0.0, scale=1.0)

            def emit_tru(o):
                for k in range(KH):
                    tru = ps1.tile([128, 128], BF16, name="tru", tag="ps")
                    nc.tensor.transpose(tru, u2[o][:, k * 128 : (k + 1) * 128], idm_sb)
                    dst = ucat[k][:, o * L : (o + 1) * L]
                    if k % 2 == 0:
                        nc.vector.tensor_copy(dst, tru)
                    else:
                        nc.scalar.activation(dst, tru, AF.Copy, bias=0.0, scale=1.0)

            emit_psu(0)
            emit_psu(1)
            emit_tru(0)
            emit_psu(2)
            emit_tru(1)
            emit_psu(3)
            emit_tru(2)
            emit_tru(3)
            emit_prelu(0)

            # ---- stage ubias: ubias[(o,y)] = sum_j' Wa[o,512,j'] h2[y,j'] ----
            psub = ps1.tile([1, NOUT * L], F32, name="ps", tag="ps")
            for o in range(NOUT):
                for c in range(KH):
                    nc.tensor.matmul(
                        psub[0:1, o * L : (o + 1) * L],
                        wab_sb[:, c * NOUT + o : c * NOUT + o + 1],
                        h2t[:, c * L : (c + 1) * L],
                        start=(c == 0),
                        stop=(c == KH - 1),
                    )
            nc.vector.tensor_copy(ubias_r, psub)
            psbias = ps1.tile([128, NOUT * L], F32, name="ps", tag="ps")
            nc.tensor.matmul(psbias, ones_r, ubias_r, start=True, stop=True)
            nc.scalar.activation(ubias_bc, psbias, AF.Copy, bias=0.0, scale=1.0)

            emit_zg_tt(4)  # vector, after its trU-copy work
            for g in range(1, 4):
                emit_prelu(g)

            # ---- steady loop: matmul pairs + drains, zg/prelu interleaved ----
            for g in range(NG):
                if g + 5 < NG:
                    emit_zg_tt(g + 5)
                if g + 4 < NG:
                    emit_prelu(g + 4)
                h1g_v = h1_all[g].rearrange("p (il k j) -> p il k j", il=G, k=KH)
                for half in range(G // 2):
                    pair = g * 2 + half
                    seeded = pair in SC_PAIRS
                    pso = ps2.tile([128, 2 * NOUT * L], F32, name="pso", tag="pso")
                    for sub in range(2):
                        il = half * 2 + sub
                        sl = pso[:, sub * NOUT * L : (sub + 1) * NOUT * L]
                        if seeded:
                            nc.tensor.matmul(sl, ones_r, ubias_r, start=True, stop=False)
                        for k in range(KH):
                            nc.tensor.matmul(
                                sl,
                                h1g_v[:, il, k],
                                ucat[k],
                                start=(k == 0 and not seeded),
                                stop=(k == KH - 1),
                            )
                    osb = outp.tile([128, 2 * NOUT * L], BF16, name="osb")
                    if seeded:
                        nc.scalar.activation(osb, pso, AF.Copy, bias=0.0, scale=1.0)
                    else:
                        nc.vector.tensor_tensor(
                            osb.rearrange("p (i f) -> p i f", i=2),
                            pso.rearrange("p (i f) -> p i f", i=2),
                            ubias_bc[:, None, :].to_broadcast((128, 2, NOUT * L)),
                            ALU.add,
                        )
                    i0 = pair * 2
                    nc.sync.dma_start(
                        out[i0 : i0 + 2, :, :].rearrange("i p f -> p i f"),
                        osb.rearrange("p (i f) -> p i f", i=2),
                    )

    bass_rust.generate_event_semaphores(nc)
    return nc


def _prep_common(W1, b1, W2, b2, Wa):
    """Host-side weight preprocessing shared by all cores."""
    import ml_dtypes

    W1s = (np.asarray(W1, np.float32) * SCALE).astype(np.float32)
    b1s = (np.asarray(b1, np.float32) * SCALE).astype(np.float32)
    W2s = (np.asarray(W2, np.float32) * SCALE).astype(np.float32)
    b2s = (np.asarray(b2, np.float32) * SCALE).astype(np.float32)
    Wa = np.asarray(Wa, np.float32)

    b1t = np.ascontiguousarray(b1s.reshape(KH, 128).T)  # [128, KH]
    b2t = np.ascontiguousarray(b2s.reshape(KH, 128).T)

    # wat[o][p, c*512+i'] = Wa[o, i', c*128+p]
    watT = Wa.transpose(0, 2, 1)[:, :, :HID]  # [o, j, i']
    wat = np.ascontiguousarray(
        watT.reshape(NOUT, KH, 128, HID).transpose(0, 2, 1, 3).reshape(NOUT, 128, KH * HID)
    ).astype(ml_dtypes.bfloat16)
    # wab[p, c*4+o] = Wa[o, 512, c*128+p]
    wab = np.ascontiguousarray(
        Wa[:, HID, :].reshape(NOUT, KH, 128).transpose(2, 1, 0).reshape(128, KH * NOUT)
    ).astype(ml_dtypes.bfloat16)
    idm = np.eye(128, dtype=np.float32).astype(ml_dtypes.bfloat16)
    return W1s, W2s, b1t, b2t, wat, wab, idm


LAST_RESULT = None


def kernel(x_const, x_dep, W1, b1, W2, b2, Wa):
    global LAST_RESULT
    import ml_dtypes
    from concourse.bass_utils import run_bass_kernel_spmd

    x_const = np.asarray(x_const, np.float32)
    x_dep = np.asarray(x_dep, np.float32)
    W1s, W2s, b1t, b2t, wat, wab, idm = _prep_common(W1, b1, W2, b2, Wa)

    if "nc" not in _CACHED:
        _CACHED["nc"] = _build_nc()
    nc = _CACHED["nc"]

    in_maps = []
    perms = []
    for core in range(NCORES):
        b, ih = core // 2, core % 2
        perm = np.concatenate(
            [
                np.arange(ih * ILOC, (ih + 1) * ILOC),
                np.arange((1 - ih) * ILOC, (2 - ih) * ILOC),
            ]
        )
        perms.append(perm)
        xcT = np.ascontiguousarray(x_const[b].T[:, perm])  # [IND, L], cols permuted
        xdT = np.ascontiguousarray(x_dep[b].T)  # [IND, L]
        w1x = np.concatenate([W1s, xcT], axis=1).astype(ml_dtypes.bfloat16)
        w2x = np.concatenate([W2s, xdT], axis=1).astype(ml_dtypes.bfloat16)
        in_maps.append(
            {
                "w1x": w1x,
                "w2x": w2x,
                "b1t": b1t,
                "b2t": b2t,
                "wat": wat,
                "wab": wab,
                "idm": idm,
            }
        )

    _tdir = _os.environ.get("KERNEL_TRACE_DIR")
    _kw = {}
    if _tdir:
        _os.makedirs(_tdir, exist_ok=True)
        _kw["tmpdir"] = _tdir
    res = run_bass_kernel_spmd(nc, in_maps, core_ids=list(range(NCORES)), **_kw)
    LAST_RESULT = res

    out_full = np.empty((B, NOUT, L, L, L), np.float32)
    for core in range(NCORES):
        b, ih = core // 2, core % 2
        perm = perms[core]
        inv = np.argsort(perm)
        core_out = np.asarray(res.results[core]["out"]).astype(np.float32)
        core_out = core_out.reshape(ILOC, L, NOUT, L).transpose(2, 0, 1, 3)
        out_full[b, :, ih * ILOC : (ih + 1) * ILOC, :, :] = core_out[:, :, inv, :]
    return out_full


# revision 20
# speedup vs baseline: 1.0672x; 1.0672x over previous
"""Trainium2 Bass kernel for nn_BiaffineSpan2WordLabeler.

Reference computation (B=4, L=128, IN=1024, H=512, NOUT=4):
    diff[b,i,j]  = x_const[b,j] - x_const[b,i]              # [B, L, L, IN]
    h1 = leaky(diff @ W1 + b1) * SCALE                      # [B, L*L, H]
    h2 = leaky(x_dep @ W2 + b2) * SCALE                     # [B, L, H]
    out[b,o,x,y] = sum_i h1b[b,x,i] Wa[o,i,j] h2[b,y,j]     # h1b = [h1, 1]

Algebraic restructurings (exact up to fp rounding):
  1. diff @ W1 = P[j] - P[i] where P = x_const @ W1; leaky applied after
     the elementwise assembly z[i,j] = P[j] - P[i] + b1.
  2. SCALE folded into W1,b1,W2,b2 host-side.
  3. Biaffine contracted as u[o,y,:] = Wa[o]·h2[y] first, then out = h1·u.

Sharding: 8 cores = (batch b = core//2) x (half of the i axis); SPMD via
host-side column permutation of x_const.

v8 — every decision below comes from measured v1-v7 trace data:
  * Input DMA: per-queue ~235GB/s, aggregate ~390GB/s, ~680ns serial
    trigger cost, fixed ~8us framework startup before the first trigger.
    So: w1x (the critical input: it gates PT -> nsneg -> all h1
    production) is split across BOTH hwdge queues and loads FIRST with
    everything else quiet; then w2x halves, then wat; tiny consts on
    gpsimd's software DGE.
  * PT and h2 as 8x 512-row matmuls in transposed orientation (213ns
    each, LDWEIGHTS hidden) + PE transpose matmuls back (fp32 107ns,
    bf16 53ns per [128,128] slab) — the 128-row chains of v4 paced at
    ~310ns/matmul and held nsneg back to ~26us.
  * b2 enters via a rank-1 ones x b2row matmul into the same PSUM group;
    b1 via per-k tensor_scalar (the 3D broadcast TT costs 2.3us).
  * zg/pts/nsneg FP32 (bf16 broadcast TT is 3x slower on DVE); zg TTs
    emitted separately from prelus (strict in-order engine queues); split
    vector {0,3,4,8,11,14} / gpsimd (rest, free after 4 const triggers).
  * ubias via transpose: 4 matmuls vs wab into [y,o], PE-transpose to
    [o,y], rank-1 broadcast matmuls to all 128 partitions.
  * Steady: [128,1024] PSUM pairs, 8 matmuls, vector TT drain (+ubias
    fused), bf16 pair out-DMA on sync.
"""

import sys

_REPO = "/opt/trn_rl_repo"
if _REPO not in sys.path:
    sys.path.insert(0, _REPO)

import os as _os

import numpy as np

B, L, IND, HID, NOUT = 4, 128, 1024, 512, 4
SCALE = 1.0 / (HID**0.25)
NCORES = 8
ILOC = 64  # i-values per core
KH = 4  # HID / 128
CIN = 8  # IND / 128
G = 4  # i-values per leaky group
NG = ILOC // G
VEC_ZG = {0, 3, 4, 8, 11, 14}  # zg groups on vector; rest on gpsimd

_CACHED = {}


def _build_nc():
    import concourse.bass as bass
    import concourse.mybir as mybir
    from concourse.tile import TileContext
    import bass_rust

    F32 = mybir.dt.float32
    BF16 = mybir.dt.bfloat16
    AF = mybir.ActivationFunctionType
    ALU = mybir.AluOpType

    nc = bass.Bass()

    # [c*128+p, 0:512] = W1'[c*128+p, :], [c*128+p, 512:640] = xcT_perm[c*128+p, :]
    w1x = nc.dram_tensor("w1x", [IND, HID + L], BF16, kind="ExternalInput")
    w2x = nc.dram_tensor("w2x", [IND, HID + L], BF16, kind="ExternalInput")
    b1t = nc.dram_tensor("b1t", [128, KH], F32, kind="ExternalInput")
    b2r = nc.dram_tensor("b2r", [1, HID], BF16, kind="ExternalInput")
    # wat[o][p, c*512 + i'] = Wa[o, i', c*128+p]   (i' < 512)
    wat = nc.dram_tensor("wat", [NOUT, 128, KH * HID], BF16, kind="ExternalInput")
    # wab[p, c*4+o] = Wa[o, 512, c*128+p]
    wab = nc.dram_tensor("wab", [128, KH * NOUT], BF16, kind="ExternalInput")
    idm = nc.dram_tensor("idm", [128, 128], F32, kind="ExternalInput")
    out = nc.dram_tensor("out", [ILOC, L, NOUT * L], BF16, kind="ExternalOutput")

    with TileContext(nc) as tc:
        with (
            tc.tile_pool(name="constp", bufs=1) as constp,
            tc.tile_pool(name="wpool", bufs=3) as wpool,
            tc.tile_pool(name="watp", bufs=2) as watp,
            tc.tile_pool(name="pers", bufs=1) as pers,
            tc.tile_pool(name="work", bufs=6) as work,
            tc.tile_pool(name="h1pool", bufs=1) as h1pool,
            tc.tile_pool(name="outp", bufs=4) as outp,
            tc.tile_pool(name="ps1", bufs=2, space="PSUM") as ps1,
            tc.tile_pool(name="ps2", bufs=3, space="PSUM") as ps2,
        ):
            # ---- input DMAs, staged: w1x halves on both hwdge queues first,
            # then w2x halves, then wat; consts via gpsimd software DGE ----
            wx_all = [wpool.tile([128, HID + L], BF16, name="wx", tag="wx", bufs=CIN) for _ in range(CIN)]
            w2_all = [wpool.tile([128, HID + L], BF16, name="w2", tag="w2", bufs=CIN) for _ in range(CIN)]
            wato_all = [
                watp.tile([128, KH * HID], BF16, name="wato", bufs=NOUT)
                for _ in range(NOUT)
            ]
            for c in range(4):
                nc.sync.dma_start(wx_all[c], w1x[c * 128 : (c + 1) * 128, :])
                nc.scalar.dma_start(wx_all[c + 4], w1x[(c + 4) * 128 : (c + 5) * 128, :])
            for c in range(4):
                nc.sync.dma_start(w2_all[c], w2x[c * 128 : (c + 1) * 128, :])
                nc.scalar.dma_start(w2_all[c + 4], w2x[(c + 4) * 128 : (c + 5) * 128, :])
            nc.sync.dma_start(wato_all[0], wat[0, :, :])
            nc.scalar.dma_start(wato_all[1], wat[1, :, :])
            nc.sync.dma_start(wato_all[2], wat[2, :, :])
            nc.scalar.dma_start(wato_all[3], wat[3, :, :])

            b1t_sb = constp.tile([128, KH], F32)
            nc.gpsimd.dma_start(b1t_sb, b1t[:, :])
            b2row = constp.tile([1, HID], BF16)
            nc.gpsimd.dma_start(b2row, b2r[:, :])
            wab_sb = constp.tile([128, KH * NOUT], BF16)
            nc.gpsimd.dma_start(wab_sb, wab[:, :])
            idf_sb = constp.tile([128, 128], F32)
            nc.gpsimd.dma_start(idf_sb, idm[:, :])
            idb_sb = constp.tile([128, 128], BF16)
            nc.vector.tensor_copy(idb_sb, idf_sb)
            ones_f = constp.tile([1, 128], F32)
            nc.vector.memset(ones_f, 1.0)
            ones_r = constp.tile([1, 128], BF16)
            nc.vector.tensor_copy(ones_r, ones_f)

            # ---- persistent intermediates ----
            ptt = pers.tile([128, HID], F32)  # P^T: [j, h]
            pts = pers.tile([128, KH * L], F32)  # P: [h', (k, j)]
            nsneg = pers.tile([128, KH * ILOC], F32)  # P[:,i] - b1: [h', (k,i)]
            h2pre = pers.tile([128, HID], BF16)  # h2: [y, h]
            h2t = pers.tile([128, KH * L], BF16)  # h2^T: [j', (c, y)]
            u2 = [pers.tile([128, HID], BF16, name=f"u2_{o}") for o in range(NOUT)]
            ubias_r = pers.tile([1, NOUT * L], BF16)
            ucat = [
                pers.tile([128, NOUT * L], BF16, name=f"ucat{k}") for k in range(KH)
            ]
            ubias_bc = pers.tile([128, NOUT * L], F32)

            # ---- stage P (transposed): PTT[j, h] = sum_in xcT[in,j] W1'[in,h] ----
            pspt = ps1.tile([128, HID], F32, name="ps", tag="ps")
            for c in range(CIN):
                nc.tensor.matmul(
                    pspt,
                    wx_all[c][:, HID : HID + L],
                    wx_all[c][:, 0:HID],
                    start=(c == 0),
                    stop=(c == CIN - 1),
                )
            nc.vector.tensor_copy(ptt, pspt)
            for k in range(KH):
                trp = ps1.tile([128, 128], F32, name="trp", tag="ps")
                nc.tensor.transpose(trp, ptt[:, k * 128 : (k + 1) * 128], idf_sb)
                nc.vector.tensor_copy(pts[:, k * L : (k + 1) * L], trp)

            pts_kj = pts.rearrange("p (k j) -> p k j", k=KH)
            for k in range(KH):
                nc.vector.tensor_scalar_sub(
                    nsneg[:, k * ILOC : (k + 1) * ILOC],
                    pts[:, k * L : k * L + ILOC],
                    b1t_sb[:, k : k + 1],
                )
            nsneg_ki = nsneg.rearrange("p (k i) -> p k i", k=KH)

            # ---- zg TTs (separate from prelus; strict in-order queues) ----
            h1_all = [
                h1pool.tile([128, G * KH * L], BF16, name=f"h1g{g}") for g in range(NG)
            ]
            zg_all = {}

            def emit_zg_tt(g):
                zg = work.tile([128, G * KH * L], F32, name="zg")
                zg_all[g] = zg
                z_eng = nc.vector if g in VEC_ZG else nc.gpsimd
                z_eng.tensor_tensor(
                    zg.rearrange("p (il k j) -> p il k j", il=G, k=KH),
                    pts_kj[:, None, :, :].to_broadcast((128, G, KH, L)),
                    nsneg_ki[:, :, g * G : (g + 1) * G]
                    .rearrange("p k i -> p i k")[:, :, :, None]
                    .to_broadcast((128, G, KH, L)),
                    ALU.subtract,
                )

            def emit_prelu(g):
                nc.scalar.activation(
                    h1_all[g], zg_all.pop(g), AF.Prelu, bias=0.0, scale=1.0, alpha=0.1
                )

            emit_zg_tt(0)  # vector
            emit_zg_tt(1)  # gpsimd
            emit_zg_tt(2)  # gpsimd
            emit_zg_tt(3)  # vector

            # ---- stage h2 (transposed): h2pre[y,h] = leaky(sum xdT W2' + b2) ----
            psh2 = ps1.tile([128, HID], F32, name="ps", tag="ps")
            for c in range(CIN):
                nc.tensor.matmul(
                    psh2,
                    w2_all[c][:, HID : HID + L],
                    w2_all[c][:, 0:HID],
                    start=(c == 0),
                    stop=False,
                )
            nc.tensor.matmul(psh2, ones_r, b2row, start=False, stop=True)
            nc.scalar.activation(h2pre, psh2, AF.Prelu, bias=0.0, scale=1.0, alpha=0.1)
            for c in range(KH):
                trh = ps1.tile([128, 128], BF16, name="trh", tag="ps")
                nc.tensor.transpose(trh, h2pre[:, c * 128 : (c + 1) * 128], idb_sb)
                nc.scalar.activation(
                    h2t[:, c * L : (c + 1) * L], trh, AF.Copy, bias=0.0, scale=1.0
                )

            # ---- stage u (transposed) + ubias ----
            def emit_psu(o):
                psu = ps1.tile([128, HID], F32, name="ps", tag="ps")
                for c in range(KH):
                    nc.tensor.matmul(
                        psu,
                        h2t[:, c * L : (c + 1) * L],
                        wato_all[o][:, c * HID : (c + 1) * HID],
                        start=(c == 0),
                        stop=(c == KH - 1),
                    )
                nc.scalar.activation(u2[o], psu, AF.Copy, bias=0.0, scale=1.0)

            def emit_tru(o):
                for k in range(KH):
                    tru = ps1.tile([128, 128], BF16, name="tru", tag="ps")
                    nc.tensor.transpose(tru, u2[o][:, k * 128 : (k + 1) * 128], idb_sb)
                    dst = ucat[k][:, o * L : (o + 1) * L]
                    if k % 2 == 0:
                        nc.vector.tensor_copy(dst, tru)
                    else:
                        nc.scalar.activation(dst, tru, AF.Copy, bias=0.0, scale=1.0)

            emit_psu(0)
            emit_psu(1)
            emit_tru(0)
            emit_psu(2)
            emit_tru(1)
            emit_psu(3)
            emit_tru(2)

            # ubias[(o,y)] = sum_j' Wa[o,512,j'] h2[y,j']
            psub = ps1.tile([1, NOUT * L], F32, name="ps", tag="ps")
            for o in range(NOUT):
                for c in range(KH):
                    nc.tensor.matmul(
                        psub[0:1, o * L : (o + 1) * L],
                        wab_sb[:, c * NOUT + o : c * NOUT + o + 1],
                        h2t[:, c * L : (c + 1) * L],
                        start=(c == 0),
                        stop=(c == KH - 1),
                    )
            emit_tru(3)
            emit_prelu(0)
            nc.vector.tensor_copy(ubias_r, psub)
            psbias = ps1.tile([128, NOUT * L], F32, name="ps", tag="ps")
            nc.tensor.matmul(psbias, ones_r, ubias_r, start=True, stop=True)
            nc.scalar.activation(ubias_bc, psbias, AF.Copy, bias=0.0, scale=1.0)

            emit_zg_tt(4)  # vector, after its trU-copy work
            for g in range(1, 4):
                emit_prelu(g)

            # ---- steady loop: matmul pairs + drains, zg/prelu interleaved ----
            for g in range(NG):
                if g + 5 < NG:
                    emit_zg_tt(g + 5)
                if g + 4 < NG:
                    emit_prelu(g + 4)
                h1g_v = h1_all[g].rearrange("p (il k j) -> p il k j", il=G, k=KH)
                for half in range(G // 2):
                    pair = g * 2 + half
                    pso = ps2.tile([128, 2 * NOUT * L], F32, name="pso", tag="pso")
                    for sub in range(2):
                        il = half * 2 + sub
                        sl = pso[:, sub * NOUT * L : (sub + 1) * NOUT * L]
                        for k in range(KH):
                            nc.tensor.matmul(
                                sl,
                                h1g_v[:, il, k],
                                ucat[k],
                                start=(k == 0),
                                stop=(k == KH - 1),
                            )
                    osb = outp.tile([128, 2 * NOUT * L], BF16, name="osb")
                    nc.vector.tensor_tensor(
                        osb.rearrange("p (i f) -> p i f", i=2),
                        pso.rearrange("p (i f) -> p i f", i=2),
                        ubias_bc[:, None, :].to_broadcast((128, 2, NOUT * L)),
                        ALU.add,
                    )
                    i0 = pair * 2
                    nc.sync.dma_start(
                        out[i0 : i0 + 2, :, :].rearrange("i p f -> p i f"),
                        osb.rearrange("p (i f) -> p i f", i=2),
                    )

    bass_rust.generate_event_semaphores(nc)
    return nc


def _prep_common(W1, b1, W2, b2, Wa):
    """Host-side weight preprocessing shared by all cores."""
    import ml_dtypes

    W1s = (np.asarray(W1, np.float32) * SCALE).astype(np.float32)
    b1s = (np.asarray(b1, np.float32) * SCALE).astype(np.float32)
    W2s = (np.asarray(W2, np.float32) * SCALE).astype(np.float32)
    b2s = (np.asarray(b2, np.float32) * SCALE).astype(np.float32)
    Wa = np.asarray(Wa, np.float32)

    b1t = np.ascontiguousarray(b1s.reshape(KH, 128).T)  # [128, KH]
    b2row = np.ascontiguousarray(b2s.reshape(1, HID)).astype(ml_dtypes.bfloat16)

    # wat[o][p, c*512+i'] = Wa[o, i', c*128+p]
    watT = Wa.transpose(0, 2, 1)[:, :, :HID]  # [o, j, i']
    wat = np.ascontiguousarray(
        watT.reshape(NOUT, KH, 128, HID).transpose(0, 2, 1, 3).reshape(NOUT, 128, KH * HID)
    ).astype(ml_dtypes.bfloat16)
    # wab[p, c*4+o] = Wa[o, 512, c*128+p]
    wab = np.ascontiguousarray(
        Wa[:, HID, :].reshape(NOUT, KH, 128).transpose(2, 1, 0).reshape(128, KH * NOUT)
    ).astype(ml_dtypes.bfloat16)
    idm = np.eye(128, dtype=np.float32)
    return W1s, W2s, b1t, b2row, wat, wab, idm


LAST_RESULT = None


def kernel(x_const, x_dep, W1, b1, W2, b2, Wa):
    global LAST_RESULT
    import ml_dtypes
    from concourse.bass_utils import run_bass_kernel_spmd

    x_const = np.asarray(x_const, np.float32)
    x_dep = np.asarray(x_dep, np.float32)
    W1s, W2s, b1t, b2row, wat, wab, idm = _prep_common(W1, b1, W2, b2, Wa)

    if "nc" not in _CACHED:
        _CACHED["nc"] = _build_nc()
    nc = _CACHED["nc"]

    in_maps = []
    perms = []
    for core in range(NCORES):
        b, ih = core // 2, core % 2
        perm = np.concatenate(
            [
                np.arange(ih * ILOC, (ih + 1) * ILOC),
                np.arange((1 - ih) * ILOC, (2 - ih) * ILOC),
            ]
        )
        perms.append(perm)
        xcT = np.ascontiguousarray(x_const[b].T[:, perm])  # [IND, L], cols permuted
        xdT = np.ascontiguousarray(x_dep[b].T)  # [IND, L]
        w1x = np.concatenate([W1s, xcT], axis=1).astype(ml_dtypes.bfloat16)
        w2x = np.concatenate([W2s, xdT], axis=1).astype(ml_dtypes.bfloat16)
        in_maps.append(
            {
                "w1x": w1x,
                "w2x": w2x,
                "b1t": b1t,
                "b2r": b2row,
                "wat": wat,
                "wab": wab,
                "idm": idm,
            }
        )

    _tdir = _os.environ.get("KERNEL_TRACE_DIR")
    _kw = {}
    if _tdir:
        _os.makedirs(_tdir, exist_ok=True)
        _kw["tmpdir"] = _tdir
    res = run_bass_kernel_spmd(nc, in_maps, core_ids=list(range(NCORES)), **_kw)
    LAST_RESULT = res

    out_full = np.empty((B, NOUT, L, L, L), np.float32)
    for core in range(NCORES):
        b, ih = core // 2, core % 2
        perm = perms[core]
        inv = np.argsort(perm)
        core_out = np.asarray(res.results[core]["out"]).astype(np.float32)
        core_out = core_out.reshape(ILOC, L, NOUT, L).transpose(2, 0, 1, 3)
        out_full[b, :, ih * ILOC : (ih + 1) * ILOC, :, :] = core_out[:, :, inv, :]
    return out_full


# revision 22
# speedup vs baseline: 1.0686x; 1.0013x over previous
"""Trainium2 Bass kernel for nn_BiaffineSpan2WordLabeler.

Reference computation (B=4, L=128, IN=1024, H=512, NOUT=4):
    diff[b,i,j]  = x_const[b,j] - x_const[b,i]              # [B, L, L, IN]
    h1 = leaky(diff @ W1 + b1) * SCALE                      # [B, L*L, H]
    h2 = leaky(x_dep @ W2 + b2) * SCALE                     # [B, L, H]
    out[b,o,x,y] = sum_i h1b[b,x,i] Wa[o,i,j] h2[b,y,j]     # h1b = [h1, 1]

Algebraic restructurings (exact up to fp rounding):
  1. diff @ W1 = P[j] - P[i] where P = x_const @ W1; leaky applied after
     the elementwise assembly z[i,j] = P[j] - P[i] + b1.
  2. SCALE folded into W1,b1,W2,b2 host-side.
  3. Biaffine contracted as u[o,y,:] = Wa[o]·h2[y] first, then out = h1·u.

Sharding: 8 cores = (batch b = core//2) x (half of the i axis); SPMD via
host-side column permutation of x_const.

v8 — every decision below comes from measured v1-v7 trace data:
  * Input DMA: per-queue ~235GB/s, aggregate ~390GB/s, ~680ns serial
    trigger cost, fixed ~8us framework startup before the first trigger.
    So: w1x (the critical input: it gates PT -> nsneg -> all h1
    production) is split across BOTH hwdge queues and loads FIRST with
    everything else quiet; then w2x halves, then wat; tiny consts on
    gpsimd's software DGE.
  * PT and h2 as 8x 512-row matmuls in transposed orientation (213ns
    each, LDWEIGHTS hidden) + PE transpose matmuls back (fp32 107ns,
    bf16 53ns per [128,128] slab) — the 128-row chains of v4 paced at
    ~310ns/matmul and held nsneg back to ~26us.
  * b2 enters via a rank-1 ones x b2row matmul into the same PSUM group;
    b1 via per-k tensor_scalar (the 3D broadcast TT costs 2.3us).
  * zg/pts/nsneg FP32 (bf16 broadcast TT is 3x slower on DVE); zg TTs
    emitted separately from prelus (strict in-order engine queues); split
    vector {0,3,4,8,11,14} / gpsimd (rest, free after 4 const triggers).
  * ubias via transpose: 4 matmuls vs wab into [y,o], PE-transpose to
    [o,y], rank-1 broadcast matmuls to all 128 partitions.
  * Steady: [128,1024] PSUM pairs, 8 matmuls, vector TT drain (+ubias
    fused), bf16 pair out-DMA on sync.
"""

import sys

_REPO = "/opt/trn_rl_repo"
if _REPO not in sys.path:
    sys.path.insert(0, _REPO)

import os as _os

import numpy as np

B, L, IND, HID, NOUT = 4, 128, 1024, 512, 4
SCALE = 1.0 / (HID**0.25)
NCORES = 8
ILOC = 64  # i-values per core
KH = 4  # HID / 128
CIN = 8  # IND / 128
G = 4  # i-values per leaky group
NG = ILOC // G
VEC_ZG = {0, 3, 4, 8, 11}  # zg groups on vector; rest on gpsimd
SC_PAIRS = {29, 30, 31}  # late pairs drained by scalar from ubias-seeded PSUM

_CACHED = {}


def _build_nc():
    import concourse.bass as bass
    import concourse.mybir as mybir
    from concourse.tile import TileContext
    import bass_rust

    F32 = mybir.dt.float32
    BF16 = mybir.dt.bfloat16
    AF = mybir.ActivationFunctionType
    ALU = mybir.AluOpType

    nc = bass.Bass()

    # [c*128+p, 0:512] = W1'[c*128+p, :], [c*128+p, 512:640] = xcT_perm[c*128+p, :]
    w1x = nc.dram_tensor("w1x", [IND, HID + L], BF16, kind="ExternalInput")
    w2x = nc.dram_tensor("w2x", [IND, HID + L], BF16, kind="ExternalInput")
    b1t = nc.dram_tensor("b1t", [128, KH], F32, kind="ExternalInput")
    b2r = nc.dram_tensor("b2r", [1, HID], BF16, kind="ExternalInput")
    # wat[o][p, c*512 + i'] = Wa[o, i', c*128+p]   (i' < 512)
    wat = nc.dram_tensor("wat", [NOUT, 128, KH * HID], BF16, kind="ExternalInput")
    # wab[p, c*4+o] = Wa[o, 512, c*128+p]
    wab = nc.dram_tensor("wab", [128, KH * NOUT], BF16, kind="ExternalInput")
    idm = nc.dram_tensor("idm", [128, 128], F32, kind="ExternalInput")
    out = nc.dram_tensor("out", [ILOC, L, NOUT * L], BF16, kind="ExternalOutput")

    with TileContext(nc) as tc:
        with (
            tc.tile_pool(name="constp", bufs=1) as constp,
            tc.tile_pool(name="wpool", bufs=3) as wpool,
            tc.tile_pool(name="watp", bufs=2) as watp,
            tc.tile_pool(name="pers", bufs=1) as pers,
            tc.tile_pool(name="work", bufs=6) as work,
            tc.tile_pool(name="h1pool", bufs=1) as h1pool,
            tc.tile_pool(name="outp", bufs=4) as outp,
            tc.tile_pool(name="ps1", bufs=2, space="PSUM") as ps1,
            tc.tile_pool(name="ps2", bufs=3, space="PSUM") as ps2,
        ):
            # ---- input DMAs, staged: w1x halves on both hwdge queues first,
            # then w2x halves, then wat; consts via gpsimd software DGE ----
            wx_all = [wpool.tile([128, HID + L], BF16, name="wx", tag="wx", bufs=CIN) for _ in range(CIN)]
            w2_all = [wpool.tile([128, HID + L], BF16, name="w2", tag="w2", bufs=CIN) for _ in range(CIN)]
            wato_all = [
                watp.tile([128, KH * HID], BF16, name="wato", bufs=NOUT)
                for _ in range(NOUT)
            ]
            for c in range(4):
                nc.sync.dma_start(wx_all[c], w1x[c * 128 : (c + 1) * 128, :])
                nc.scalar.dma_start(wx_all[c + 4], w1x[(c + 4) * 128 : (c + 5) * 128, :])
            for c in range(4):
                nc.sync.dma_start(w2_all[c], w2x[c * 128 : (c + 1) * 128, :])
                nc.scalar.dma_start(w2_all[c + 4], w2x[(c + 4) * 128 : (c + 5) * 128, :])
            nc.sync.dma_start(wato_all[0], wat[0, :, :])
            nc.scalar.dma_start(wato_all[1], wat[1, :, :])
            nc.sync.dma_start(wato_all[2], wat[2, :, :])
            nc.scalar.dma_start(wato_all[3], wat[3, :, :])

            b1t_sb = constp.tile([128, KH], F32)
            nc.gpsimd.dma_start(b1t_sb, b1t[:, :])
            b2row = constp.tile([1, HID], BF16)
            nc.gpsimd.dma_start(b2row, b2r[:, :])
            wab_sb = constp.tile([128, KH * NOUT], BF16)
            nc.gpsimd.dma_start(wab_sb, wab[:, :])
            idf_sb = constp.tile([128, 128], F32)
            nc.gpsimd.dma_start(idf_sb, idm[:, :])
            idb_sb = constp.tile([128, 128], BF16)
            nc.vector.tensor_copy(idb_sb, idf_sb)
            ones_f = constp.tile([1, 128], F32)
            nc.vector.memset(ones_f, 1.0)
            ones_r = constp.tile([1, 128], BF16)
            nc.vector.tensor_copy(ones_r, ones_f)

            # ---- persistent intermediates ----
            ptt = pers.tile([128, HID], F32)  # P^T: [j, h]
            pts = pers.tile([128, KH * L], F32)  # P: [h', (k, j)]
            nsneg = pers.tile([128, KH * ILOC], F32)  # P[:,i] - b1: [h', (k,i)]
            h2pre = pers.tile([128, HID], BF16)  # h2: [y, h]
            h2t = pers.tile([128, KH * L], BF16)  # h2^T: [j', (c, y)]
            u2 = [pers.tile([128, HID], BF16, name=f"u2_{o}") for o in range(NOUT)]
            ubias_r = pers.tile([1, NOUT * L], BF16)
            ucat = [
                pers.tile([128, NOUT * L], BF16, name=f"ucat{k}") for k in range(KH)
            ]
            ubias_bc = pers.tile([128, NOUT * L], F32)

            # ---- stage P (transposed): PTT[j, h] = sum_in xcT[in,j] W1'[in,h] ----
            pspt = ps1.tile([128, HID], F32, name="ps", tag="ps")
            for c in range(CIN):
                nc.tensor.matmul(
                    pspt,
                    wx_all[c][:, HID : HID + L],
                    wx_all[c][:, 0:HID],
                    start=(c == 0),
                    stop=(c == CIN - 1),
                )
            nc.vector.tensor_copy(ptt, pspt)
            for k in range(KH):
                trp = ps1.tile([128, 128], F32, name="trp", tag="ps")
                nc.tensor.transpose(trp, ptt[:, k * 128 : (k + 1) * 128], idf_sb)
                nc.vector.tensor_copy(pts[:, k * L : (k + 1) * L], trp)

            pts_kj = pts.rearrange("p (k j) -> p k j", k=KH)
            for k in range(KH):
                nc.vector.tensor_scalar_sub(
                    nsneg[:, k * ILOC : (k + 1) * ILOC],
                    pts[:, k * L : k * L + ILOC],
                    b1t_sb[:, k : k + 1],
                )
            nsneg_ki = nsneg.rearrange("p (k i) -> p k i", k=KH)

            # ---- zg TTs (separate from prelus; strict in-order queues) ----
            h1_all = [
                h1pool.tile([128, G * KH * L], BF16, name=f"h1g{g}") for g in range(NG)
            ]
            zg_all = {}

            def emit_zg_tt(g):
                zg = work.tile([128, G * KH * L], F32, name="zg")
                zg_all[g] = zg
                z_eng = nc.vector if g in VEC_ZG else nc.gpsimd
                z_eng.tensor_tensor(
                    zg.rearrange("p (il k j) -> p il k j", il=G, k=KH),
                    pts_kj[:, None, :, :].to_broadcast((128, G, KH, L)),
                    nsneg_ki[:, :, g * G : (g + 1) * G]
                    .rearrange("p k i -> p i k")[:, :, :, None]
                    .to_broadcast((128, G, KH, L)),
                    ALU.subtract,
                )

            def emit_prelu(g):
                nc.scalar.activation(
                    h1_all[g], zg_all.pop(g), AF.Prelu, bias=0.0, scale=1.0, alpha=0.1
                )

            emit_zg_tt(0)  # vector
            emit_zg_tt(1)  # gpsimd
            emit_zg_tt(2)  # gpsimd
            emit_zg_tt(3)  # vector

            # ---- stage h2 (transposed): h2pre[y,h] = leaky(sum xdT W2' + b2) ----
            psh2 = ps1.tile([128, HID], F32, name="ps", tag="ps")
            for c in range(CIN):
                nc.tensor.matmul(
                    psh2,
                    w2_all[c][:, HID : HID + L],
                    w2_all[c][:, 0:HID],
                    start=(c == 0),
                    stop=False,
                )
            nc.tensor.matmul(psh2, ones_r, b2row, start=False, stop=True)
            nc.scalar.activation(h2pre, psh2, AF.Prelu, bias=0.0, scale=1.0, alpha=0.1)
            for c in range(KH):
                trh = ps1.tile([128, 128], BF16, name="trh", tag="ps")
                nc.tensor.transpose(trh, h2pre[:, c * 128 : (c + 1) * 128], idb_sb)
                nc.scalar.activation(
                    h2t[:, c * L : (c + 1) * L], trh, AF.Copy, bias=0.0, scale=1.0
                )

            # ---- stage u (transposed) + ubias ----
            def emit_psu(o):
                psu = ps1.tile([128, HID], F32, name="ps", tag="ps")
                for c in range(KH):
                    nc.tensor.matmul(
                        psu,
                        h2t[:, c * L : (c + 1) * L],
                        wato_all[o][:, c * HID : (c + 1) * HID],
                        start=(c == 0),
                        stop=(c == KH - 1),
                    )
                nc.scalar.activation(u2[o], psu, AF.Copy, bias=0.0, scale=1.0)

            def emit_tru(o):
                for k in range(KH):
                    tru = ps1.tile([128, 128], BF16, name="tru", tag="ps")
                    nc.tensor.transpose(tru, u2[o][:, k * 128 : (k + 1) * 128], idb_sb)
                    dst = ucat[k][:, o * L : (o + 1) * L]
                    if k % 2 == 0:
                        nc.vector.tensor_copy(dst, tru)
                    else:
                        nc.scalar.activation(dst, tru, AF.Copy, bias=0.0, scale=1.0)

            emit_psu(0)
            emit_psu(1)
            emit_tru(0)
            emit_psu(2)
            emit_tru(1)
            emit_psu(3)
            emit_tru(2)

            # ubias[(o,y)] = sum_j' Wa[o,512,j'] h2[y,j']
            psub = ps1.tile([1, NOUT * L], F32, name="ps", tag="ps")
            for o in range(NOUT):
                for c in range(KH):
                    nc.tensor.matmul(
                        psub[0:1, o * L : (o + 1) * L],
                        wab_sb[:, c * NOUT + o : c * NOUT + o + 1],
                        h2t[:, c * L : (c + 1) * L],
                        start=(c == 0),
                        stop=(c == KH - 1),
                    )
            emit_tru(3)
            emit_prelu(0)
            nc.vector.tensor_copy(ubias_r, psub)
            psbias = ps1.tile([128, NOUT * L], F32, name="ps", tag="ps")
            nc.tensor.matmul(psbias, ones_r, ubias_r, start=True, stop=True)
            nc.scalar.activation(ubias_bc, psbias, AF.Copy, bias=0.0, scale=1.0)

            emit_zg_tt(4)  # vector, after its trU-copy work
            for g in range(1, 4):
                emit_prelu(g)

            # ---- steady loop: matmul pairs + drains, zg/prelu interleaved ----
            for g in range(NG):
                if g + 5 < NG:
                    emit_zg_tt(g + 5)
                if g + 4 < NG:
                    emit_prelu(g + 4)
                h1g_v = h1_all[g].rearrange("p (il k j) -> p il k j", il=G, k=KH)
                for half in range(G // 2):
                    pair = g * 2 + half
                    seeded = pair in SC_PAIRS
                    pso = ps2.tile([128, 2 * NOUT * L], F32, name="pso", tag="pso")
                    for sub in range(2):
                        il = half * 2 + sub
                        sl = pso[:, sub * NOUT * L : (sub + 1) * NOUT * L]
                        if seeded:
                            nc.tensor.matmul(sl, ones_r, ubias_r, start=True, stop=False)
                        for k in range(KH):
                            nc.tensor.matmul(
                                sl,
                                h1g_v[:, il, k],
                                ucat[k],
                                start=(k == 0 and not seeded),
                                stop=(k == KH - 1),
                            )
                    osb = outp.tile([128, 2 * NOUT * L], BF16, name="osb")
                    if seeded:
                        nc.scalar.activation(osb, pso, AF.Copy, bias=0.0, scale=1.0)
                    else:
                        nc.vector.tensor_tensor(
                            osb.rearrange("p (i f) -> p i f", i=2),
                            pso.rearrange("p (i f) -> p i f", i=2),
                            ubias_bc[:, None, :].to_broadcast((128, 2, NOUT * L)),
                            ALU.add,
                        )
                    i0 = pair * 2
                    nc.sync.dma_start(
                        out[i0 : i0 + 2, :, :].rearrange("i p f -> p i f"),
                        osb.rearrange("p (i f) -> p i f", i=2),
                    )

    bass_rust.generate_event_semaphores(nc)
    return nc


def _prep_common(W1, b1, W2, b2, Wa):
    """Host-side weight preprocessing shared by all cores."""
    import ml_dtypes

    W1s = (np.asarray(W1, np.float32) * SCALE).astype(np.float32)
    b1s = (np.asarray(b1, np.float32) * SCALE).astype(np.float32)
    W2s = (np.asarray(W2, np.float32) * SCALE).astype(np.float32)
    b2s = (np.asarray(b2, np.float32) * SCALE).astype(np.float32)
    Wa = np.asarray(Wa, np.float32)

    b1t = np.ascontiguousarray(b1s.reshape(KH, 128).T)  # [128, KH]
    b2row = np.ascontiguousarray(b2s.reshape(1, HID)).astype(ml_dtypes.bfloat16)

    # wat[o][p, c*512+i'] = Wa[o, i', c*128+p]
    watT = Wa.transpose(0, 2, 1)[:, :, :HID]  # [o, j, i']
    wat = np.ascontiguousarray(
        watT.reshape(NOUT, KH, 128, HID).transpose(0, 2, 1, 3).reshape(NOUT, 128, KH * HID)
    ).astype(ml_dtypes.bfloat16)
    # wab[p, c*4+o] = Wa[o, 512, c*128+p]
    wab = np.ascontiguousarray(
        Wa[:, HID, :].reshape(NOUT, KH, 128).transpose(2, 1, 0).reshape(128, KH * NOUT)
    ).astype(ml_dtypes.bfloat16)
    idm = np.eye(128, dtype=np.float32)
    return W1s, W2s, b1t, b2row, wat, wab, idm


LAST_RESULT = None


def kernel(x_const, x_dep, W1, b1, W2, b2, Wa):
    global LAST_RESULT
    import ml_dtypes
    from concourse.bass_utils import run_bass_kernel_spmd

    x_const = np.asarray(x_const, np.float32)
    x_dep = np.asarray(x_dep, np.float32)
    W1s, W2s, b1t, b2row, wat, wab, idm = _prep_common(W1, b1, W2, b2, Wa)

    if "nc" not in _CACHED:
        _CACHED["nc"] = _build_nc()
    nc = _CACHED["nc"]

    in_maps = []
    perms = []
    for core in range(NCORES):
        b, ih = core // 2, core % 2
        perm = np.concatenate(
            [
                np.arange(ih * ILOC, (ih + 1) * ILOC),
                np.arange((1 - ih) * ILOC, (2 - ih) * ILOC),
            ]
        )
        perms.append(perm)
        xcT = np.ascontiguousarray(x_const[b].T[:, perm])  # [IND, L], cols permuted
        xdT = np.ascontiguousarray(x_dep[b].T)  # [IND, L]
        w1x = np.concatenate([W1s, xcT], axis=1).astype(ml_dtypes.bfloat16)
        w2x = np.concatenate([W2s, xdT], axis=1).astype(ml_dtypes.bfloat16)
        in_maps.append(
            {
                "w1x": w1x,
                "w2x": w2x,
                "b1t": b1t,
                "b2r": b2row,
                "wat": wat,
                "wab": wab,
                "idm": idm,
            }
        )

    _tdir = _os.environ.get("KERNEL_TRACE_DIR")
    _kw = {}
    if _tdir:
        _os.makedirs(_tdir, exist_ok=True)
        _kw["tmpdir"] = _tdir
    res = run_bass_kernel_spmd(nc, in_maps, core_ids=list(range(NCORES)), **_kw)
    LAST_RESULT = res

    out_full = np.empty((B, NOUT, L, L, L), np.float32)
    for core in range(NCORES):
        b, ih = core // 2, core % 2
        perm = perms[core]
        inv = np.argsort(perm)
        core_out = np.asarray(res.results[core]["out"]).astype(np.float32)
        core_out = core_out.reshape(ILOC, L, NOUT, L).transpose(2, 0, 1, 3)
        out_full[b, :, ih * ILOC : (ih + 1) * ILOC, :, :] = core_out[:, :, inv, :]
    return out_full
